# revision 2
# baseline (speedup 1.0000x reference)
"""Tensor-parallel causal multi-head attention for Trainium2 (8 NeuronCores).

Problem: B=1, S=4096, D=1024, 16 heads x d_head=64, causal, fp32.

Sharding: heads are split 2-per-core across 8 cores (tensor parallel).  Each
core computes its 2 heads end-to-end (QKV projections, scores, softmax,
z = attn @ v, and its row-shard of the W_O projection) and writes a full
[D, S] partial output (fp16); the all-reduce over cores is done host-side by
summing the 8 partials.

v2 design notes (vs the 349us baseline):
  - The PE chains back-to-back matmuls at pure compute cost (216ns per
    512-col bf16 MM, LoadStationary fully hidden) but any idle gap resets
    the clock p-state (2.4GHz -> 1.2GHz for ~3-4us).  So the whole schedule
    is built to keep the PE stream dependency-free and gapless:
      * attention kp-loop runs DESCENDING so the 4 diagonal (masked) blocks
        come first and their exp->mask(Pool)->z chain hides under the
        full-width blocks that follow
      * z matmuls trail scores by 2 blocks (depth-2 pipeline) so the ACT
        exp latency never gates the PE
      * W_O matmuls for chunk p-1 are interleaved INTO chunk p's kp-loop
      * projections for chunk p+1 run right after attention(p)
  - v is computed directly transposed ([position, head-dim]) by swapping
    matmul roles: stationary = xT block, moving = W_V columns.  This kills
    the PE transposes, the identity, and the scalar-engine vT copy.
  - everything on the PE is bf16 (et/exp output, v3, zn, weights); PSUM
    accumulation stays fp32.  Causal masking multiplies the 4 diagonal
    128-strips by a [128,128] triangle on the Pool engine after exp.
  - softmax denominators accumulate in PSUM row 64 via a ones-column
    appended to v3; normalization = DVE reciprocal -> Pool
    partition_broadcast -> DVE multiply (PE not involved).
  - output partials are written fp16 (half the HBM writes; adds ~0.1% err).
"""

import os

import ml_dtypes
import numpy as np

import concourse.bass as bass
import concourse.mybir as mybir
import concourse.tile as tile
from concourse import bacc
from concourse import bass_utils

# Problem dims (hardcoded per the harness contract).
D = 1024          # d_model
S = 4096          # sequence length
NH = 16           # total heads
DH = 64           # head dim
N_CORES = 8
HPC = NH // N_CORES   # heads per core = 2
F = HPC * DH          # per-core feature slice of W_O = 128
P = 128               # SBUF partitions
QC = 512              # q chunk (matmul moving free dim)
NQ = S // QC          # 8
KP = 128              # key-position chunk (PSUM partition dim)
DCH = D // P          # 8 chunks of d_model
VW = DH + 1           # (legacy) v-columns per head incl. ones column
V3W = 128             # v3 block: col 0 = ones, cols 64..127 = v dims

F32 = mybir.dt.float32
F16 = mybir.dt.float16
BF16 = mybir.dt.bfloat16
EXP = mybir.ActivationFunctionType.Exp


def _build_program(dbg=False):
    nc = bacc.Bacc("TRN2", target_bir_lowering=False, debug=False)

    xT_d = nc.dram_tensor("xT", [D, S], BF16, kind="ExternalInput")
    # w[kqv] are host-swizzled to [p, dc*F] so each loads as one DMA with
    # 2KB-contiguous partition rows
    wk_d = nc.dram_tensor("wkT", [P, DCH * F], BF16, kind="ExternalInput")
    wq_d = nc.dram_tensor("wqT", [P, DCH * F], BF16, kind="ExternalInput")
    wv_d = nc.dram_tensor("wvT", [P, DCH * F], BF16, kind="ExternalInput")
    wo_d = nc.dram_tensor("woT", [F, D], BF16, kind="ExternalInput")
    mk_d = nc.dram_tensor("masks", [P, P], BF16, kind="ExternalInput")
    on_d = nc.dram_tensor("ones", [P, 2 * S // P], BF16, kind="ExternalInput")
    out_d = nc.dram_tensor("outT", [D, S], F16, kind="ExternalOutput")
    if dbg:
        dbg_kq_d = nc.dram_tensor("dbg_kq", [P, 2 * S], BF16, kind="ExternalOutput")
        dbg_v3_d = nc.dram_tensor("dbg_v3", [P, (S // P) * 2 * V3W], BF16, kind="ExternalOutput")
        dbg_zn_d = nc.dram_tensor("dbg_zn", [P, S], BF16, kind="ExternalOutput")

    with tile.TileContext(nc) as tc:
        with (
            tc.tile_pool(name="const", bufs=1) as cpool,
            tc.tile_pool(name="work", bufs=3) as wpool,
            tc.tile_pool(name="psum", bufs=3, space="PSUM") as ppool,
        ):
            # ---- persistent SBUF state ----
            wk_sb = cpool.tile([P, DCH, F], BF16)
            wq_sb = cpool.tile([P, DCH, F], BF16)
            wv_sb = cpool.tile([P, DCH, F], BF16)
            wo_sb = cpool.tile([P, D], BF16)        # [f, d]
            mk_sb = cpool.tile([P, P], BF16)        # keep col >= row
            ones_t = cpool.tile([P, 2 * S // P], BF16)
            kT_sb = cpool.tile([P, S], BF16)
            qT_sb = cpool.tile([P, S], BF16)
            # v3 block per (kp, head): [128] cols; col 0 = ones column (so
            # the softmax denominator accumulates on PSUM partition 0, where
            # the fast reciprocal ucode works), cols 64..127 = v dims (so z
            # lands on partitions 64..127, aligned for the in-place
            # normalize multiply).  Host swaps the head order in W_O.
            v3_sb = cpool.tile([P, (S // P) * 2 * V3W], BF16)
            # dedicated et tiles for the 4 diagonal blocks per chunk: their
            # [0, n0) prefix is zeroed once and never rewritten, so z matmuls
            # can always run full-width with a single clean accumulation
            # group per PSUM bank (interleaved same-bank groups corrupt).
            diag_et = [
                cpool.tile([P, 2 * QC], BF16, name=f"diag_et{j}")
                for j in range(4)
            ]
            for de in diag_et:
                nc.vector.memset(de[:], 0.0)

            nc.sync.dma_start(wk_sb[:], wk_d[:].rearrange("p (c f) -> p c f", f=F))
            nc.sync.dma_start(wq_sb[:], wq_d[:].rearrange("p (c f) -> p c f", f=F))
            nc.sync.dma_start(wv_sb[:], wv_d[:].rearrange("p (c f) -> p c f", f=F))
            nc.sync.dma_start(mk_sb[:], mk_d[:])
            nc.sync.dma_start(ones_t[:], on_d[:])
            nc.vector.memset(v3_sb[:], 0.0)
            v3v = v3_sb.rearrange("p (t c) -> p t c", c=V3W)
            nc.vector.tensor_copy(v3v[:, :, 0:1], ones_t[:, :, None])

            xt_tiles = {}

            def emit_x_dma(pc):
                for dc in range(DCH):
                    xt = wpool.tile([P, QC], BF16, tag="xt", bufs=2 * DCH)
                    nc.sync.dma_start(
                        xt[:],
                        xT_d[:][dc * P : (dc + 1) * P, pc * QC : (pc + 1) * QC],
                    )
                    xt_tiles[(pc, dc)] = xt

            def emit_proj(pc):
                """K/Q projections + direct-transposed V for p-chunk pc.
                kq: stationary = W chunk, moving = x tile  -> [feature, pos]
                v:  stationary = x 128-block, moving = W_V -> [pos, feature]
                """
                xts = [xt_tiles.pop((pc, dc)) for dc in range(DCH)]
                sl = slice(pc * QC, (pc + 1) * QC)
                # k fully before q: the k copy overlaps the q matmuls, so its
                # ring slot frees before v_ps needs it
                k_ps = ppool.tile([P, QC], F32, tag="b1", bufs=2)
                for dc in range(DCH):
                    nc.tensor.matmul(
                        k_ps[:], wk_sb[:, dc, :], xts[dc][:],
                        start=(dc == 0), stop=(dc == DCH - 1),
                    )
                nc.vector.tensor_copy(kT_sb[:, sl], k_ps[:])
                q_ps = ppool.tile([P, QC], F32, tag="b1", bufs=2)
                for dc in range(DCH):
                    nc.tensor.matmul(
                        q_ps[:], wq_sb[:, dc, :], xts[dc][:],
                        start=(dc == 0), stop=(dc == DCH - 1),
                    )
                nc.vector.tensor_copy(qT_sb[:, sl], q_ps[:])
                # v quarters: one SEQUENTIAL accumulation group per 128-pos
                # block (interleaved groups within one PSUM bank corrupt);
                # descending t so attention's first (highest-kp) z block gets
                # its v3 columns first.
                v_ps = ppool.tile([P, QC], F32, tag="b1", bufs=2)
                for t in range(3, -1, -1):
                    for dc in range(DCH):
                        nc.tensor.matmul(
                            v_ps[:, t * P : (t + 1) * P],
                            xts[dc][:, t * P : (t + 1) * P],
                            wv_sb[:, dc, :],
                            start=(dc == 0), stop=(dc == DCH - 1),
                        )
                    base = (4 * pc + t) * 2 * V3W
                    nc.vector.tensor_copy(
                        v3_sb[:, base + DH : base + V3W],
                        v_ps[:, t * P : t * P + DH],
                    )
                    nc.vector.tensor_copy(
                        v3_sb[:, base + V3W + DH : base + 2 * V3W],
                        v_ps[:, t * P + DH : (t + 1) * P],
                    )

            def emit_attention(qc, interleave):
                """Scores/exp/z for q-chunk qc.  kp runs DESCENDING so the 4
                diagonal (masked) blocks come first; z trails scores by two
                blocks so exp (and Pool masking) never gates the PE.
                `interleave` is a list of closures (W_O work for qc-1) spread
                through the loop."""
                z0 = ppool.tile([P, QC], F32, tag="zb", bufs=2)
                z1 = ppool.tile([P, QC], F32, tag="zb", bufs=2)
                nkp = 4 * qc + 4
                kps = list(range(nkp - 1, -1, -1))

                first_z = [True]

                def emit_z(kp, et, n0):
                    st = first_z[0]
                    first_z[0] = False
                    sp = (kp == 0)
                    vbase = kp * 2 * V3W
                    nc.tensor.matmul(
                        z0[:], v3_sb[:, vbase : vbase + V3W], et[:, 0:QC],
                        start=st, stop=sp,
                    )
                    nc.tensor.matmul(
                        z1[:], v3_sb[:, vbase + V3W : vbase + 2 * V3W],
                        et[:, QC : 2 * QC],
                        start=st, stop=sp,
                    )

                pending = []  # [(kp, et, n0), ...] z matmuls not yet emitted
                ilv = list(interleave)
                n_ilv = 0
                half = len(kps) // 2
                for idx, kp in enumerate(kps):
                    # W_O steps live in the second half of the loop: the
                    # chunk-boundary DVE backlog (copies + norm) must drain
                    # before their ob copies can run
                    while n_ilv < len(ilv) * max(0, idx - half) // max(1, len(kps) - half):
                        ilv[n_ilv]()
                        n_ilv += 1
                    j = kp - 4 * qc
                    n0 = max(0, j) * P
                    sc = ppool.tile([P, 2 * QC], F32, tag="sc", bufs=2)
                    ksl = slice(kp * P, (kp + 1) * P)
                    qn = slice(qc * QC + n0, (qc + 1) * QC)
                    nc.tensor.matmul(
                        sc[:, n0:QC],
                        kT_sb[0:DH, ksl], qT_sb[0:DH, qn],
                        start=True, stop=True,
                    )
                    nc.tensor.matmul(
                        sc[:, QC + n0 : 2 * QC],
                        kT_sb[DH : 2 * DH, ksl], qT_sb[DH : 2 * DH, qn],
                        start=True, stop=True,
                    )
                    if j >= 1:
                        et = diag_et[j]
                    else:
                        et = wpool.tile([P, 2 * QC], BF16, tag="et", bufs=6)
                    if n0 == 0:
                        nc.scalar.activation(et[:], sc[:], EXP, scale=0.125)
                    else:
                        ev = et.rearrange("p (h q) -> p h q", h=2)[:, :, n0:QC]
                        sv = sc.rearrange("p (h q) -> p h q", h=2)[:, :, n0:QC]
                        nc.scalar.activation(ev, sv, EXP, scale=0.125)
                    if j >= 0:
                        # causal triangle lives in the 128-wide strip
                        # [n0, n0+128); one small multiply masks both heads
                        e3 = et.rearrange("p (h q) -> p h q", h=2)[
                            :, :, n0 : n0 + P
                        ]
                        mb = mk_sb[:, None, :].to_broadcast((P, 2, P))
                        nc.vector.tensor_tensor(e3, e3, mb, mybir.AluOpType.mult)
                    pending.append((kp, et, n0))
                    if len(pending) > 2:
                        emit_z(*pending.pop(0))
                for args in pending:
                    emit_z(*args)
                while n_ilv < len(ilv):
                    ilv[n_ilv]()
                    n_ilv += 1
                return z0, z1

            def emit_zcopy(z0, z1):
                """Plain copies PSUM -> SBUF so the z banks free immediately,
                then approx reciprocals (fast single DVE op) and Pool
                broadcasts.  The multiplies are deferred (emit_norm) so the
                next chunk's kq/v3 copies aren't queued behind this chain."""
                out = []
                for zp in (z0, z1):
                    zu = wpool.tile([P, QC], F32, tag="zu", bufs=4)
                    nc.vector.tensor_copy(zu[:], zp[:])
                    rc = wpool.tile([1, QC], F32, tag="rc", bufs=2)
                    nc.vector.reciprocal_approx_fast(rc[:], zu[0:1, :])
                    bc = wpool.tile([P, QC], F32, tag="bc", bufs=2)
                    nc.gpsimd.partition_broadcast(bc[:], rc[:], channels=P)
                    out.append((zu, bc))
                return out

            def emit_norm(qc, zs):
                """zn = z * (1/denom), head 1 shifted to partitions 64..127
                via an SBUF DMA."""
                zn = wpool.tile([P, QC], BF16, tag="zn")
                for h, (zu, bc) in enumerate(zs):
                    if h == 0:
                        # head0 dims sit on partitions 64..127; multiply in
                        # place (Pool, all-SBUF) -> zn rows 64:128
                        nc.vector.tensor_mul(
                            out=zn[DH:P, :], in0=zu[DH:P, :], in1=bc[DH:P, :]
                        )
                    else:
                        zt = wpool.tile([P, QC], BF16, tag="zt")
                        nc.vector.tensor_mul(
                            out=zt[DH:P, :], in0=zu[DH:P, :], in1=bc[DH:P, :]
                        )
                        # head1 -> partitions 0..63 (DMA shifts partitions)
                        nc.sync.dma_start(zn[0:DH, :], zt[DH:P, :])
                return zn

            def make_wo_step(qc, zn, dc):
                qsl = slice(qc * QC, (qc + 1) * QC)

                def step():
                    wop = ppool.tile([P, QC], F32, tag="b1", bufs=2)
                    nc.tensor.matmul(
                        wop[:], wo_sb[:, dc * P : (dc + 1) * P], zn[:],
                        start=True, stop=True,
                    )
                    ob = wpool.tile([P, QC], F16, tag="ob", bufs=6)
                    nc.vector.tensor_copy(ob[:], wop[:])
                    nc.sync.dma_start(out_d[:][dc * P : (dc + 1) * P, qsl], ob[:])

                return step

            # ---- schedule ----
            emit_x_dma(0)
            emit_x_dma(1)
            nc.sync.dma_start(wo_sb[:], wo_d[:])
            emit_proj(0)
            prev = None  # (qc, zn) whose W_O is pending
            for pc in range(NQ):
                ilv = (
                    [make_wo_step(prev[0], prev[1], dc) for dc in range(DCH)]
                    if prev is not None
                    else []
                )
                z0, z1 = emit_attention(pc, ilv)
                if pc + 2 < NQ:
                    emit_x_dma(pc + 2)
                if pc + 1 < NQ:
                    emit_proj(pc + 1)
                zs = emit_zcopy(z0, z1)
                zn = emit_norm(pc, zs)
                if dbg:
                    nc.sync.dma_start(dbg_zn_d[:][:, pc * QC : (pc + 1) * QC], zn[:])
                prev = (pc, zn)
            for dc in range(DCH):
                make_wo_step(prev[0], prev[1], dc)()
            if dbg:
                nc.sync.dma_start(dbg_kq_d[:][:, 0:S], kT_sb[:])
                nc.sync.dma_start(dbg_kq_d[:][:, S : 2 * S], qT_sb[:])
                nc.sync.dma_start(dbg_v3_d[:], v3_sb[:])

    nc.compile()  # bacc passes: DCE, register allocation, nop fusion
    return nc


_LAST_RESULTS = None  # BassKernelResults of the most recent run (for test.py)


def kernel(x, W_K, W_Q, W_V, W_O):
    global _LAST_RESULTS
    x = np.asarray(x, dtype=np.float32)
    W_K = np.asarray(W_K, dtype=np.float32)
    W_Q = np.asarray(W_Q, dtype=np.float32)
    W_V = np.asarray(W_V, dtype=np.float32)
    W_O = np.asarray(W_O, dtype=np.float32)
    B = x.shape[0]
    assert x.shape == (B, S, D) and B == 1

    bf16 = ml_dtypes.bfloat16
    xT = np.ascontiguousarray(x[0].T).astype(bf16)   # [D, S]
    # causal keep-mask for the diagonal 128-strip: keep col >= row
    i = np.arange(P)
    mask = (i[None, :] >= i[:, None]).astype(bf16)   # [128, 128]
    ones = np.ones((P, 2 * S // P), bf16)

    in_maps = []
    for c in range(N_CORES):
        hs = slice(HPC * c, HPC * (c + 1))
        def swz(w):
            # [H, dh, D] -> wT [D, F] -> [p, dc*F] (partition-major chunks)
            wT = w[hs].transpose(2, 0, 1).reshape(D, F)
            return np.ascontiguousarray(
                wT.reshape(DCH, P, F).transpose(1, 0, 2).reshape(P, DCH * F)
            ).astype(bf16)
        wkT, wqT, wvT = swz(W_K), swz(W_Q), swz(W_V)
        wo_c = W_O[:, F * c : F * (c + 1)]
        wo_swapped = np.concatenate([wo_c[:, DH:F], wo_c[:, 0:DH]], axis=1)
        woT = np.ascontiguousarray(wo_swapped.T).astype(bf16)  # [F, D]
        in_maps.append(
            {"xT": xT, "wkT": wkT, "wqT": wqT, "wvT": wvT, "woT": woT,
             "masks": mask, "ones": ones}
        )

    nc = _build_program(dbg=os.environ.get("KERNEL_DBG", "0") == "1")
    trace = os.environ.get("KERNEL_TRACE", "0") == "1"
    res = bass_utils.run_bass_kernel_spmd(
        nc, in_maps, core_ids=list(range(N_CORES)), trace=trace
    )
    _LAST_RESULTS = res

    acc = np.zeros((D, S), dtype=np.float32)
    for r in res.results:
        acc += np.asarray(r["outT"], dtype=np.float32)
    return np.ascontiguousarray(acc.T)[None]      # [1, S, D] fp32
